# revision 5
# baseline (speedup 1.0000x reference)
"""Causal single-head attention (B=8, T=2048, C=512, D=64) on 8 trn2 NeuronCores.

Sharding: data-parallel over batch — core b computes attention for x[b].
Small projection weights (Wq, Wk, Wv) are replicated to every core.

Per-core dataflow (all in SBUF, fp32 storage, float32r matmuls):
  1. x [T, C] -> PE-transpose -> xT [C, T]            (64x [128,128] transposes)
  2. QT [64, T] = Wq.T @ xT, KT likewise              (contraction over C)
  3. V  [T, Cout] tiles = xT_chunk.T @ Wv_chunk       (natural [t, c] layout)
  4. per query slice s (512 wide), per key chunk j (128):
       ST  [tk=128, tq=512] = KT_chunk.T-style matmul (lhsT=KT[:,j], rhs=QT[:,s])
       E   = exp(0.125 * ST)       (ACT, PSUM->SBUF; no max-subtraction needed:
                                    scores ~ N(0,1), exp can't overflow)
       E  *= tri_mask              (diagonal-band chunks only)
       Z  [1, 512]   += ones.T @ E                    (row sums via PE)
       out[tq=128, c=512] += E[:, k-slice].T @ V_j    (lands in natural layout)
     then rz = 1/Z, transposed to columns via PE, folded into the final
     PSUM->SBUF copy as a per-partition activation scale.
"""

import os
import sys

if "/opt/trn_rl_repo" not in sys.path:
    sys.path.insert(0, "/opt/trn_rl_repo")

import numpy as np

import concourse.bass as bass
import concourse.tile as tile
from concourse import bacc, mybir
from concourse.bass import ts

B, T, C_IN, C_OUT, D = 8, 2048, 512, 512, 64
NT = T // 128  # 16 key chunks / t tiles
NC = C_IN // 128  # 4 c_in chunks
NS = T // 512  # 4 query slices
F32 = mybir.dt.float32
F32R = mybir.dt.float32r

USE_F32R = os.environ.get("KERNEL_F32R", "1") == "1"
MMDT = F32R if USE_F32R else F32

last_result = None  # BassKernelResults of the most recent run (for test harness)


def _emit(tc):
    nc = tc.nc

    xb = nc.dram_tensor("xb", [T, C_IN], F32, kind="ExternalInput").ap()
    wq = nc.dram_tensor("wq", [C_IN, D], MMDT, kind="ExternalInput").ap()
    wk = nc.dram_tensor("wk", [C_IN, D], MMDT, kind="ExternalInput").ap()
    wv = nc.dram_tensor("wv", [C_IN, C_OUT], MMDT, kind="ExternalInput").ap()
    masks = nc.dram_tensor("masks", [128, 2048], MMDT, kind="ExternalInput").ap()
    ident = nc.dram_tensor("ident", [128, 128], F32, kind="ExternalInput").ap()
    ones = nc.dram_tensor("ones", [128, 1], MMDT, kind="ExternalInput").ap()
    out = nc.dram_tensor("out", [T, C_OUT], F32, kind="ExternalOutput").ap()

    with (
        tc.tile_pool(name="persist", bufs=1) as pp,
        tc.tile_pool(name="epool", bufs=3) as ep,
        tc.tile_pool(name="opool", bufs=2) as outp,
        tc.tile_pool(name="rzpool", bufs=2) as rzp,
        tc.tile_pool(name="stp", bufs=2, space="PSUM") as stp,
        tc.tile_pool(name="op", bufs=4, space="PSUM") as op,
        tc.tile_pool(name="zp", bufs=1, space="PSUM") as zp,
        tc.tile_pool(name="ztrp", bufs=1, space="PSUM") as ztrp,
    ):
        # ---- persistent SBUF tensors ----
        x_sb = pp.tile([128, NT * C_IN], F32, tag="x")  # t-tile i at ts(i, 512)
        xt_sb = pp.tile([128, NC * T], MMDT, tag="xt")  # c-chunk j at ts(j, 2048)
        qt_sb = pp.tile([D, T], MMDT, tag="qt")
        kt_sb = pp.tile([D, T], MMDT, tag="kt")
        v_sb = pp.tile([128, NT * C_OUT], MMDT, tag="v")  # tk-tile j at ts(j, 512)
        wq_sb = pp.tile([128, NC * D], MMDT, tag="wq")
        wk_sb = pp.tile([128, NC * D], MMDT, tag="wk")
        wv_sb = pp.tile([128, NC * C_OUT], MMDT, tag="wv")
        masks_sb = pp.tile([128, 2048], MMDT, tag="masks")
        id_sb = pp.tile([128, 128], F32, tag="ident")
        ones_sb = pp.tile([128, 1], MMDT, tag="ones")

        # ---- input DMAs ----
        nc.sync.dma_start(
            x_sb.rearrange("p (i c) -> p i c", c=C_IN),
            xb.rearrange("(i p) c -> p i c", p=128),
        )
        nc.sync.dma_start(
            wq_sb.rearrange("p (j d) -> p j d", d=D),
            wq.rearrange("(j p) d -> p j d", p=128),
        )
        nc.sync.dma_start(
            wk_sb.rearrange("p (j d) -> p j d", d=D),
            wk.rearrange("(j p) d -> p j d", p=128),
        )
        nc.sync.dma_start(
            wv_sb.rearrange("p (j c) -> p j c", c=C_OUT),
            wv.rearrange("(j p) c -> p j c", p=128),
        )
        nc.sync.dma_start(masks_sb[:], masks)
        nc.sync.dma_start(id_sb[:], ident)
        nc.sync.dma_start(ones_sb[:], ones)

        # ---- 1. transpose x -> xT ----
        for j in range(NC):  # c chunk
            for g in range(NT // 4):  # t group of 512
                xtr = stp.tile([128, 512], F32, tag="st", name="xtr")
                for u in range(4):
                    i = 4 * g + u  # t tile
                    nc.tensor.transpose(
                        xtr[:, ts(u, 128)],
                        x_sb[:, 512 * i + 128 * j : 512 * i + 128 * (j + 1)],
                        id_sb[:],
                    )
                nc.vector.tensor_copy(
                    xt_sb[:, 2048 * j + 512 * g : 2048 * j + 512 * (g + 1)], xtr[:]
                )

        # ---- 2. Q/K projections: QT[64, T], KT[64, T] ----
        for s in range(NS):
            q_ps = stp.tile([D, 512], F32, tag="st", name="q_ps")
            for j in range(NC):
                nc.tensor.matmul(
                    q_ps[:],
                    wq_sb[:, ts(j, D)],
                    xt_sb[:, 2048 * j + 512 * s : 2048 * j + 512 * (s + 1)],
                    start=(j == 0),
                    stop=(j == NC - 1),
                )
            nc.vector.tensor_copy(qt_sb[:, ts(s, 512)], q_ps[:])
            k_ps = stp.tile([D, 512], F32, tag="st", name="k_ps")
            for j in range(NC):
                nc.tensor.matmul(
                    k_ps[:],
                    wk_sb[:, ts(j, D)],
                    xt_sb[:, 2048 * j + 512 * s : 2048 * j + 512 * (s + 1)],
                    start=(j == 0),
                    stop=(j == NC - 1),
                )
            nc.vector.tensor_copy(kt_sb[:, ts(s, 512)], k_ps[:])

        # ---- 3. V projection: V[t, c] tiles ----
        for i in range(NT):
            v_ps = stp.tile([128, 512], F32, tag="st", name="v_ps")
            for j in range(NC):
                nc.tensor.matmul(
                    v_ps[:],
                    xt_sb[:, 2048 * j + 128 * i : 2048 * j + 128 * (i + 1)],
                    wv_sb[:, ts(j, 512)],
                    start=(j == 0),
                    stop=(j == NC - 1),
                )
            nc.vector.tensor_copy(v_sb[:, ts(i, 512)], v_ps[:])

        # ---- 4. attention ----
        def emit_st(s, j):
            st_ps = stp.tile([128, 512], F32, tag="st", name="st_ps")
            nc.tensor.matmul(
                st_ps[:],
                kt_sb[:, ts(j, 128)],
                qt_sb[:, ts(s, 512)],
                start=True,
                stop=True,
            )
            return st_ps

        for s in range(NS):
            nj = 4 * s + 4
            o_ps = [op.tile([128, 512], F32, tag="o", name=f"o_ps{k}") for k in range(4)]
            z_ps = zp.tile([1, 512], F32, name="z_ps")
            st_tiles = {0: emit_st(s, 0)}
            for j in range(nj):
                if j + 1 < nj:
                    # software-pipeline: issue next ST before consuming this one
                    st_tiles[j + 1] = emit_st(s, j + 1)
                st_ps = st_tiles.pop(j)
                e = ep.tile([128, 512], MMDT, name="e")
                nc.scalar.activation(
                    e[:], st_ps[:], mybir.ActivationFunctionType.Exp, scale=0.125
                )
                if j >= 4 * s:
                    nc.vector.tensor_mul(e[:], e[:], masks_sb[:, ts(j - 4 * s, 512)])
                nc.tensor.matmul(
                    z_ps[:],
                    ones_sb[:],
                    e[:],
                    start=(j == 0),
                    stop=(j == nj - 1),
                )
                for k in range(4):
                    m = 4 * s + k
                    if j <= m:
                        nc.tensor.matmul(
                            o_ps[k][:],
                            e[:, ts(k, 128)],
                            v_sb[:, ts(j, 512)],
                            start=(j == 0),
                            stop=(j == m),
                        )
            # 1/Z, transposed into per-partition columns
            rz = rzp.tile([1, 512], F32, tag="rz", name="rz")
            nc.vector.reciprocal(rz[:], z_ps[:])
            rzt_ps = ztrp.tile([128, 4], F32, name="rzt_ps")
            for k in range(4):
                nc.tensor.transpose(
                    rzt_ps[:, k : k + 1], rz[0:1, ts(k, 128)], id_sb[0:1, 0:1]
                )
            rzt = rzp.tile([128, 4], F32, tag="rzt", name="rzt")
            nc.vector.tensor_copy(rzt[:], rzt_ps[:])
            for k in range(4):
                o_sb = outp.tile([128, 512], F32, name="o_sb")
                nc.scalar.activation(
                    o_sb[:],
                    o_ps[k][:],
                    mybir.ActivationFunctionType.Copy,
                    scale=rzt[:, k : k + 1],
                )
                r0 = 512 * s + 128 * k
                nc.sync.dma_start(out[r0 : r0 + 128, :], o_sb[:])


def build_nc():
    nc = bacc.Bacc(
        "TRN2",
        target_bir_lowering=False,
        debug=False,
        enable_asserts=False,
        num_devices=B,
    )
    with tile.TileContext(nc) as tc:
        _emit(tc)
    nc.compile()
    return nc


def host_constants():
    r = np.arange(4)[:, None, None]
    p = np.arange(128)[None, :, None]
    f = np.arange(512)[None, None, :]
    masks = (p <= f - 128 * r).astype(np.float32)  # [4, 128, 512]
    masks = masks.transpose(1, 0, 2).reshape(128, 2048)
    ident = np.eye(128, dtype=np.float32)
    ones = np.ones((128, 1), dtype=np.float32)
    return masks, ident, ones


_nc_cache = {}


def _install_ntff_hook():
    """Provide antenv.axon_hooks (absent in this image) so that
    run_bass_kernel_spmd(trace=True) can capture NTFF profiles via the
    axon ctypes hook from trn_agent_boot."""
    import types

    if "antenv.axon_hooks" in sys.modules:
        return
    mod = types.ModuleType("antenv.axon_hooks")
    holder = [None]
    mod.set_axon_ntff_profile_hook = lambda h: holder.__setitem__(0, h)
    mod.get_axon_ntff_profile_hook = lambda: holder[0]
    sys.modules["antenv.axon_hooks"] = mod
    try:
        from trn_agent_boot.trn_boot import _ntff_profile_via_ctypes

        holder[0] = _ntff_profile_via_ctypes("/opt/axon/libaxon_pjrt.so")
    except Exception as e:  # degrade to no tracing
        print(f"ntff hook install failed: {e}", file=sys.stderr)


def kernel(x, Wq, Wk, Wv):
    from concourse import bass_utils

    x = np.ascontiguousarray(np.asarray(x, dtype=np.float32))
    Wq = np.ascontiguousarray(np.asarray(Wq, dtype=np.float32))
    Wk = np.ascontiguousarray(np.asarray(Wk, dtype=np.float32))
    Wv = np.ascontiguousarray(np.asarray(Wv, dtype=np.float32))
    assert x.shape == (B, T, C_IN), x.shape

    if "nc" not in _nc_cache:
        _nc_cache["nc"] = build_nc()
    nc = _nc_cache["nc"]

    masks, ident, ones = host_constants()
    in_maps = [
        {
            "xb": x[b],
            "wq": Wq,
            "wk": Wk,
            "wv": Wv,
            "masks": masks,
            "ident": ident,
            "ones": ones,
        }
        for b in range(B)
    ]
    trace = os.environ.get("KERNEL_TRACE", "0") == "1"
    if trace:
        _install_ntff_hook()
    res = bass_utils.run_bass_kernel_spmd(
        nc, in_maps, core_ids=list(range(B)), trace=trace
    )
    global last_result
    last_result = res
    return np.stack([r["out"] for r in res.results], axis=0)


# revision 24
# speedup vs baseline: 1.5421x; 1.5421x over previous
"""Causal single-head attention (B=8, T=2048, C=512, D=64) on 8 trn2 NeuronCores.

Sharding: data-parallel over batch — core b computes the full causal attention
for x[b]; the small projection weights are replicated to every core. No
collectives are needed, and the final output is gathered on the host by
stacking the 8 per-core results.

Host-side prep (layout only, no FLOPs): x[b] is passed pre-transposed as
xT [C, T] (saves 64 PE transposes per core), Wq/Wk are zero-padded to
[C, 128] and fused into one wqk tensor (K=64 matmuls stream the moving
operand at half rate on trn2 — padding the contraction to 128 with zeros
doubles ST matmul throughput), and the small constants (causal triangle,
ones, identity scalar) ride in one blob to minimize DMA issue ops.

Per-core dataflow (fp32 storage, float32r matmuls, all resident in SBUF):
  1. QT/KT [128, T] = wqk_chunk.T @ xT_chunk   (rows 64:128 are zeros)
     V [t, c] tiles  = xT_chunk.T @ Wv_chunk   (natural layout for step 3)
  2. per query slice s (512 wide), per key chunk j (128):
       ST [tk=128, tq=512] = matmul(lhsT=KT[:, j], rhs=QT[:, s])  (PSUM)
       E  = exp(0.125 * ST)    ACT, PSUM->SBUF, restricted to the causally
                               valid columns; band chunks multiply the
                               [128,128] diagonal block by the triangle mask.
                               No max-subtraction: scores ~ N(0,1), exp of a
                               few units cannot overflow fp32.
       Z[tq] += ones.T @ E     row sums on the PE (1 matmul per chunk)
       out[tq=128, c=512] += matmul(lhsT=E[:, k*128:...], rhs=V_j)
     The second matmul lands the output in natural [t, c] layout directly,
     so no output transposes are needed. Outputs are copied out of PSUM
     unnormalized as soon as each 128-row block's accumulation finishes
     (frees the PSUM bank early), then scaled by 1/Z (transposed to a
     per-partition column via a tiny PE transpose, reciprocal on DVE) and
     DMA'd per 128-row block.

Performance notes (measured on trn2 via NTFF profiles):
  - float32r runs 1 cycle/column only when the operand APs are strided
    (a fully contiguous AP or a K=64 operand drops to 2 cycles/column) —
    hence the zero-padded K and the deliberate 516/132-wide tile padding.
  - ST matmuls are software-pipelined DEPTH=3 ahead so the PE never waits
    on the ACT exp; input DMAs are batched and ordered by first use across
    both HWDGE queues.
"""

import os
import sys

if "/opt/trn_rl_repo" not in sys.path:
    sys.path.insert(0, "/opt/trn_rl_repo")

import numpy as np

import concourse.tile as tile
from concourse import bacc, mybir
from concourse.bass import ts

B, T, C_IN, C_OUT, D = 8, 2048, 512, 512, 64
NT = T // 128  # 16 key chunks / t tiles
NC = C_IN // 128  # 4 c_in chunks
NS = T // 512  # 4 query slices
F32 = mybir.dt.float32
F32R = mybir.dt.float32r

USE_F32R = os.environ.get("KERNEL_F32R", "1") == "1"
MMDT = F32R if USE_F32R else F32

last_result = None  # BassKernelResults of the most recent run (for test harness)


def _emit(tc):
    nc = tc.nc

    xt = nc.dram_tensor("xt", [C_IN, T], MMDT, kind="ExternalInput").ap()
    wqk = nc.dram_tensor("wqk", [C_IN, 256], MMDT, kind="ExternalInput").ap()
    wv = nc.dram_tensor("wv", [C_IN, C_OUT], MMDT, kind="ExternalInput").ap()
    # consts blob: [:, 0:128] tri, [:, 128:260] ones, [0, 260] identity scalar
    consts = nc.dram_tensor("consts", [128, 261], MMDT, kind="ExternalInput").ap()
    out = nc.dram_tensor("out", [T, C_OUT], F32, kind="ExternalOutput").ap()

    with (
        tc.tile_pool(name="persist", bufs=1) as pp,
        tc.tile_pool(name="epool", bufs=4) as ep,
        tc.tile_pool(name="opool", bufs=2) as outp,
        tc.tile_pool(name="rzp", bufs=2) as rzp,
        tc.tile_pool(name="stp", bufs=3, space="PSUM") as stp,
        tc.tile_pool(name="op", bufs=4, space="PSUM") as op,
        tc.tile_pool(name="zp", bufs=1, space="PSUM") as zp,
    ):
        # ---- persistent SBUF tensors ----
        xt_sb = pp.tile([128, NC * T], MMDT, tag="xt")  # c-chunk j at ts(j, 2048)
        qt_sb = pp.tile([128, T], MMDT, tag="qt")
        kt_sb = pp.tile([128, T], MMDT, tag="kt")
        v_sb = pp.tile([128, NT * C_OUT], MMDT, tag="v")  # tk-tile j at ts(j, 512)
        wqk_sb = pp.tile([128, NC * 256], MMDT, tag="wqk")
        wv_sb = pp.tile([128, NC * C_OUT], MMDT, tag="wv")
        consts_sb = pp.tile([128, 261], MMDT, tag="consts")
        tri_sb = consts_sb[:, 0:128]
        ones_sb = consts_sb[:, 128:260]
        id32_sb = consts_sb[0:1, 260:261].bitcast(F32)

        # ---- input DMAs: batched, ordered by first-use time ----
        nc.sync.dma_start(xt_sb[:, 0:512], xt[0:128, 0:512])
        nc.scalar.dma_start(
            wqk_sb.rearrange("p (j d) -> p j d", d=256),
            wqk.rearrange("(j p) d -> p j d", p=128),
        )
        nc.scalar.dma_start(consts_sb[:], consts)
        nc.sync.dma_start(
            xt_sb.rearrange("p (j tt) -> p j tt", tt=2048)[:, 1:4, 0:512],
            xt[128:512, 0:512].rearrange("(j p) t -> p j t", p=128),
        )
        nc.sync.dma_start(
            wv_sb.rearrange("p (j c) -> p j c", c=C_OUT),
            wv.rearrange("(j p) c -> p j c", p=128),
        )
        for j in range(NC):
            eng = nc.sync if j % 2 == 0 else nc.scalar
            eng.dma_start(
                xt_sb[:, 2048 * j + 512 : 2048 * (j + 1)],
                xt[128 * j : 128 * (j + 1), 512:2048],
            )

        # ---- projections, per t-group g ----
        for g in range(4):
            s = g
            q_ps = stp.tile([128, 512], F32, tag="st", name="q_ps")
            for j in range(NC):
                nc.tensor.matmul(
                    q_ps[:],
                    wqk_sb[:, 256 * j : 256 * j + 128],
                    xt_sb[:, 2048 * j + 512 * s : 2048 * j + 512 * (s + 1)],
                    start=(j == 0),
                    stop=(j == NC - 1),
                )
            nc.vector.tensor_copy(qt_sb[:, ts(s, 512)], q_ps[:])
            k_ps = stp.tile([128, 512], F32, tag="st", name="k_ps")
            for j in range(NC):
                nc.tensor.matmul(
                    k_ps[:],
                    wqk_sb[:, 256 * j + 128 : 256 * (j + 1)],
                    xt_sb[:, 2048 * j + 512 * s : 2048 * j + 512 * (s + 1)],
                    start=(j == 0),
                    stop=(j == NC - 1),
                )
            nc.vector.tensor_copy(kt_sb[:, ts(s, 512)], k_ps[:])
            for i in range(4 * g, 4 * g + 4):
                v_ps = stp.tile([128, 512], F32, tag="st", name="v_ps")
                for j in range(NC):
                    nc.tensor.matmul(
                        v_ps[:],
                        xt_sb[:, 2048 * j + 128 * i : 2048 * j + 128 * (i + 1)],
                        wv_sb[:, ts(j, 512)],
                        start=(j == 0),
                        stop=(j == NC - 1),
                    )
                nc.vector.tensor_copy(v_sb[:, ts(i, 512)], v_ps[:])

        # ---- attention ----
        def emit_st(s, j):
            st_ps = stp.tile([128, 512], F32, tag="st", name="st_ps")
            nc.tensor.matmul(
                st_ps[:],
                kt_sb[:, ts(j, 128)],
                qt_sb[:, ts(s, 512)],
                start=True,
                stop=True,
            )
            return st_ps

        DEPTH = 3
        pend = {0: {j: emit_st(0, j) for j in range(DEPTH)}}
        o_bigs = {}
        for s in range(NS):
            nj = 4 * s + 4
            st_tiles = pend.pop(s)
            o_ps = [
                op.tile([128, 512], F32, tag="o", name=f"o_ps{k}") for k in range(4)
            ]
            z_ps = zp.tile([128, 512], F32, name="z_ps")
            o_bigs[s] = outp.tile([128, 2048], F32, name="o_big")
            for j in range(nj):
                jn = j + DEPTH
                if jn < nj:
                    st_tiles[jn] = emit_st(s, jn)
                elif s + 1 < NS and jn - nj < DEPTH:
                    pend.setdefault(s + 1, {})[jn - nj] = emit_st(s + 1, jn - nj)
                st_ps = st_tiles.pop(j)
                e = ep.tile([128, 516], MMDT, name="e")
                r = j - 4 * s  # band index; valid key rows start at column 128*r
                lo = 128 * r if r >= 0 else 0
                nc.scalar.activation(
                    e[:, lo:512],
                    st_ps[:, lo:512],
                    mybir.ActivationFunctionType.Exp,
                    scale=0.125,
                )
                if r >= 0:
                    nc.vector.tensor_mul(e[:, ts(r, 128)], e[:, ts(r, 128)], tri_sb[:])
                nc.tensor.matmul(
                    z_ps[:, lo:512],
                    ones_sb[:, 0:128],
                    e[:, lo:512],
                    start=(j == 0),
                    stop=(j == nj - 1),
                )
                o_big = o_bigs[s]
                for k in range(4):
                    m = 4 * s + k
                    if j <= m:
                        nc.tensor.matmul(
                            o_ps[k][:],
                            e[:, ts(k, 128)],
                            v_sb[:, ts(j, 512)],
                            start=(j == 0),
                            stop=(j == m),
                        )
                        if j == m:
                            # accumulation done: copy out unnormalized now so
                            # the PSUM bank frees before the 1/Z chain finishes
                            nc.scalar.copy(o_big[:, ts(k, 512)], o_ps[k][:])
            # 1/Z: copy row to SBUF (ACT), transpose to columns (PE), recip (DVE)
            z_sb = rzp.tile([1, 512], F32, tag="z", name="z_sb")
            nc.scalar.copy(z_sb[:], z_ps[0:1, :])
            rzt_ps = stp.tile([128, 4], F32, tag="st", name="rzt_ps")
            for k in range(4):
                nc.tensor.transpose(
                    rzt_ps[:, k : k + 1],
                    z_sb[0:1, ts(k, 128)],
                    id32_sb[:],
                )
            rzt = rzp.tile([128, 4], F32, tag="rzt", name="rzt")
            nc.vector.reciprocal(rzt[:], rzt_ps[:])
            for k in range(4):
                nc.vector.tensor_scalar_mul(
                    o_big[:, ts(k, 512)], o_big[:, ts(k, 512)], rzt[:, k : k + 1]
                )
                r0 = 512 * s + 128 * k
                nc.sync.dma_start(out[r0 : r0 + 128, :], o_big[:, ts(k, 512)])


def build_nc():
    nc = bacc.Bacc(
        "TRN2",
        target_bir_lowering=False,
        debug=False,
        enable_asserts=False,
        num_devices=B,
    )
    with tile.TileContext(nc) as tc:
        _emit(tc)
    nc.compile()
    return nc


def host_constants():
    """consts blob: [:, 0:128] causal triangle, [:, 128:260] ones,
    [0, 260] the 1x1 identity used by the tiny Z transposes."""
    p = np.arange(128)[:, None]
    f = np.arange(128)[None, :]
    consts = np.zeros((128, 261), dtype=np.float32)
    consts[:, 0:128] = (p <= f).astype(np.float32)
    consts[:, 128:260] = 1.0
    consts[0, 260] = 1.0
    return consts


_nc_cache = {}


def _install_ntff_hook():
    """Provide antenv.axon_hooks (absent in this image) so that
    run_bass_kernel_spmd(trace=True) can capture NTFF profiles via the
    axon ctypes hook from trn_agent_boot."""
    import types

    if "antenv.axon_hooks" in sys.modules:
        return
    mod = types.ModuleType("antenv.axon_hooks")
    holder = [None]
    mod.set_axon_ntff_profile_hook = lambda h: holder.__setitem__(0, h)
    mod.get_axon_ntff_profile_hook = lambda: holder[0]
    sys.modules["antenv.axon_hooks"] = mod
    try:
        from trn_agent_boot.trn_boot import _ntff_profile_via_ctypes

        holder[0] = _ntff_profile_via_ctypes("/opt/axon/libaxon_pjrt.so")
    except Exception as e:  # degrade to no tracing
        print(f"ntff hook install failed: {e}", file=sys.stderr)


def kernel(x, Wq, Wk, Wv):
    from concourse import bass_utils

    x = np.ascontiguousarray(np.asarray(x, dtype=np.float32))
    Wq = np.ascontiguousarray(np.asarray(Wq, dtype=np.float32))
    Wk = np.ascontiguousarray(np.asarray(Wk, dtype=np.float32))
    Wv = np.ascontiguousarray(np.asarray(Wv, dtype=np.float32))
    assert x.shape == (B, T, C_IN), x.shape

    if "nc" not in _nc_cache:
        _nc_cache["nc"] = build_nc()
    nc = _nc_cache["nc"]

    consts = host_constants()
    z = np.zeros_like(Wq)
    wqk = np.ascontiguousarray(np.concatenate([Wq, z, Wk, z], axis=1))
    in_maps = [
        {
            "xt": np.ascontiguousarray(x[b].T),
            "wqk": wqk,
            "wv": Wv,
            "consts": consts,
        }
        for b in range(B)
    ]
    trace = os.environ.get("KERNEL_TRACE", "0") == "1"
    if trace:
        _install_ntff_hook()
    res = bass_utils.run_bass_kernel_spmd(
        nc, in_maps, core_ids=list(range(B)), trace=trace
    )
    global last_result
    last_result = res
    return np.stack([r["out"] for r in res.results], axis=0)


# revision 28
# speedup vs baseline: 1.5656x; 1.0153x over previous
"""Causal single-head attention (B=8, T=2048, C=512, D=64) on 8 trn2 NeuronCores.

Sharding: data-parallel over batch — core b computes the full causal attention
for x[b]; the small projection weights are replicated to every core. No
collectives are needed, and the final output is gathered on the host by
stacking the 8 per-core results.

Host-side prep (layout only, no FLOPs): x[b] is passed pre-transposed as
xT [C, T] (saves 64 PE transposes per core), Wq/Wk are zero-padded to
[C, 128] and fused into one wqk tensor (K=64 matmuls stream the moving
operand at half rate on trn2 — padding the contraction to 128 with zeros
doubles ST matmul throughput), and the small constants (causal triangle,
ones, identity scalar) ride in one blob to minimize DMA issue ops.

Per-core dataflow (fp32 storage, float32r matmuls, all resident in SBUF):
  1. QT/KT [128, T] = wqk_chunk.T @ xT_chunk   (rows 64:128 are zeros)
     V [t, c] tiles  = xT_chunk.T @ Wv_chunk   (natural layout for step 3)
  2. per query slice s (512 wide), per key chunk j (128):
       ST [tk=128, tq=512] = matmul(lhsT=KT[:, j], rhs=QT[:, s])  (PSUM)
       E  = exp(0.125 * ST)    ACT, PSUM->SBUF, restricted to the causally
                               valid columns; band chunks multiply the
                               [128,128] diagonal block by the triangle mask.
                               No max-subtraction: scores ~ N(0,1), exp of a
                               few units cannot overflow fp32.
       Z[tq] += ones.T @ E     row sums on the PE (1 matmul per chunk)
       out[tq=128, c=512] += matmul(lhsT=E[:, k*128:...], rhs=V_j)
     The second matmul lands the output in natural [t, c] layout directly,
     so no output transposes are needed. Outputs are copied out of PSUM
     unnormalized as soon as each 128-row block's accumulation finishes
     (frees the PSUM bank early), then scaled by 1/Z (transposed to a
     per-partition column via a tiny PE transpose, reciprocal on DVE) and
     DMA'd per 128-row block.

Performance notes (measured on trn2 via NTFF profiles):
  - float32r runs 1 cycle/column only when the operand APs are strided
    (a fully contiguous AP or a K=64 operand drops to 2 cycles/column) —
    hence the zero-padded K and the deliberate 516/132-wide tile padding.
  - ST matmuls are software-pipelined DEPTH=3 ahead so the PE never waits
    on the ACT exp; input DMAs are batched and ordered by first use across
    both HWDGE queues.
"""

import os
import sys

if "/opt/trn_rl_repo" not in sys.path:
    sys.path.insert(0, "/opt/trn_rl_repo")

import numpy as np

import concourse.tile as tile
from concourse import bacc, mybir
from concourse.bass import ts

B, T, C_IN, C_OUT, D = 8, 2048, 512, 512, 64
NT = T // 128  # 16 key chunks / t tiles
NC = C_IN // 128  # 4 c_in chunks
NS = T // 512  # 4 query slices
F32 = mybir.dt.float32
F32R = mybir.dt.float32r

USE_F32R = os.environ.get("KERNEL_F32R", "1") == "1"
MMDT = F32R if USE_F32R else F32

last_result = None  # BassKernelResults of the most recent run (for test harness)


def _emit(tc):
    nc = tc.nc

    xt = nc.dram_tensor("xt", [C_IN, T], MMDT, kind="ExternalInput").ap()
    wqk = nc.dram_tensor("wqk", [C_IN, 256], MMDT, kind="ExternalInput").ap()
    wv = nc.dram_tensor("wv", [C_IN, C_OUT], MMDT, kind="ExternalInput").ap()
    # consts blob: [:, 0:128] tri, [:, 128:260] ones, [0, 260] identity scalar
    consts = nc.dram_tensor("consts", [128, 261], MMDT, kind="ExternalInput").ap()
    out = nc.dram_tensor("out", [T, C_OUT], F32, kind="ExternalOutput").ap()

    with (
        tc.tile_pool(name="persist", bufs=1) as pp,
        tc.tile_pool(name="epool", bufs=4) as ep,
        tc.tile_pool(name="opool", bufs=2) as outp,
        tc.tile_pool(name="rzp", bufs=2) as rzp,
        tc.tile_pool(name="stp", bufs=3, space="PSUM") as stp,
        tc.tile_pool(name="op", bufs=4, space="PSUM") as op,
        tc.tile_pool(name="zp", bufs=1, space="PSUM") as zp,
    ):
        # ---- persistent SBUF tensors ----
        xt_sb = pp.tile([128, NC * T], MMDT, tag="xt")  # c-chunk j at ts(j, 2048)
        qt_sb = pp.tile([128, T], MMDT, tag="qt")
        kt_sb = pp.tile([128, T], MMDT, tag="kt")
        v_sb = pp.tile([128, NT * C_OUT], MMDT, tag="v")  # tk-tile j at ts(j, 512)
        wqk_sb = pp.tile([128, NC * 256], MMDT, tag="wqk")
        wv_sb = pp.tile([128, NC * C_OUT], MMDT, tag="wv")
        consts_sb = pp.tile([128, 261], MMDT, tag="consts")
        tri_sb = consts_sb[:, 0:128]
        ones_sb = consts_sb[:, 128:260]
        id32_sb = consts_sb[0:1, 260:261].bitcast(F32)

        # ---- input DMAs: batched, ordered by first-use time ----
        nc.sync.dma_start(xt_sb[:, 0:512], xt[0:128, 0:512])
        nc.scalar.dma_start(
            wqk_sb.rearrange("p (j d) -> p j d", d=256),
            wqk.rearrange("(j p) d -> p j d", p=128),
        )
        nc.sync.dma_start(xt_sb[:, 4096:4608], xt[256:384, 0:512])
        nc.scalar.dma_start(xt_sb[:, 2048:2560], xt[128:256, 0:512])
        nc.sync.dma_start(wv_sb[:, 0:512], wv[0:128, :])
        nc.scalar.dma_start(xt_sb[:, 6144:6656], xt[384:512, 0:512])
        nc.sync.dma_start(wv_sb[:, 512:1024], wv[128:256, :])
        nc.sync.dma_start(wv_sb[:, 1024:1536], wv[256:384, :])
        nc.sync.dma_start(wv_sb[:, 1536:2048], wv[384:512, :])
        nc.scalar.dma_start(consts_sb[:], consts)
        for j in range(NC):
            eng = nc.sync if j % 2 == 0 else nc.scalar
            eng.dma_start(
                xt_sb[:, 2048 * j + 512 : 2048 * (j + 1)],
                xt[128 * j : 128 * (j + 1), 512:2048],
            )

        # ---- projections, per t-group g ----
        for g in range(4):
            s = g
            q_ps = stp.tile([128, 512], F32, tag="st", name="q_ps")
            for j in range(NC):
                nc.tensor.matmul(
                    q_ps[:],
                    wqk_sb[:, 256 * j : 256 * j + 128],
                    xt_sb[:, 2048 * j + 512 * s : 2048 * j + 512 * (s + 1)],
                    start=(j == 0),
                    stop=(j == NC - 1),
                )
            nc.vector.tensor_copy(qt_sb[:, ts(s, 512)], q_ps[:])
            k_ps = stp.tile([128, 512], F32, tag="st", name="k_ps")
            for j in range(NC):
                nc.tensor.matmul(
                    k_ps[:],
                    wqk_sb[:, 256 * j + 128 : 256 * (j + 1)],
                    xt_sb[:, 2048 * j + 512 * s : 2048 * j + 512 * (s + 1)],
                    start=(j == 0),
                    stop=(j == NC - 1),
                )
            nc.vector.tensor_copy(kt_sb[:, ts(s, 512)], k_ps[:])
            for i in range(4 * g, 4 * g + 4):
                v_ps = stp.tile([128, 512], F32, tag="st", name="v_ps")
                for j in range(NC):
                    nc.tensor.matmul(
                        v_ps[:],
                        xt_sb[:, 2048 * j + 128 * i : 2048 * j + 128 * (i + 1)],
                        wv_sb[:, ts(j, 512)],
                        start=(j == 0),
                        stop=(j == NC - 1),
                    )
                nc.vector.tensor_copy(v_sb[:, ts(i, 512)], v_ps[:])

        # ---- attention ----
        def emit_st(s, j):
            st_ps = stp.tile([128, 512], F32, tag="st", name="st_ps")
            nc.tensor.matmul(
                st_ps[:],
                kt_sb[:, ts(j, 128)],
                qt_sb[:, ts(s, 512)],
                start=True,
                stop=True,
            )
            return st_ps

        DEPTH = 3
        pend = {0: {j: emit_st(0, j) for j in range(DEPTH)}}
        o_bigs = {}
        for s in range(NS):
            nj = 4 * s + 4
            st_tiles = pend.pop(s)
            o_ps = [
                op.tile([128, 512], F32, tag="o", name=f"o_ps{k}") for k in range(4)
            ]
            z_ps = zp.tile([1, 512], F32, name="z_ps")
            o_bigs[s] = outp.tile([128, 2048], F32, name="o_big")
            for j in range(nj):
                jn = j + DEPTH
                if jn < nj:
                    st_tiles[jn] = emit_st(s, jn)
                elif s + 1 < NS and jn - nj < DEPTH:
                    pend.setdefault(s + 1, {})[jn - nj] = emit_st(s + 1, jn - nj)
                st_ps = st_tiles.pop(j)
                e = ep.tile([128, 516], MMDT, name="e")
                r = j - 4 * s  # band index; valid key rows start at column 128*r
                lo = 128 * r if r >= 0 else 0
                nc.scalar.activation(
                    e[:, lo:512],
                    st_ps[:, lo:512],
                    mybir.ActivationFunctionType.Exp,
                    scale=0.125,
                )
                if r >= 0:
                    nc.vector.tensor_mul(e[:, ts(r, 128)], e[:, ts(r, 128)], tri_sb[:])
                nc.tensor.matmul(
                    z_ps[0:1, lo:512],
                    ones_sb[:, 0:1],
                    e[:, lo:512],
                    start=(j == 0),
                    stop=(j == nj - 1),
                )
                o_big = o_bigs[s]
                for k in range(4):
                    m = 4 * s + k
                    if j <= m:
                        nc.tensor.matmul(
                            o_ps[k][:],
                            e[:, ts(k, 128)],
                            v_sb[:, ts(j, 512)],
                            start=(j == 0),
                            stop=(j == m),
                        )
                        if j == m:
                            # accumulation done: copy out unnormalized now so
                            # the PSUM bank frees before the 1/Z chain finishes
                            nc.scalar.copy(o_big[:, ts(k, 512)], o_ps[k][:])
            # 1/Z: copy row to SBUF (ACT), transpose to columns (PE), recip (DVE)
            z_sb = rzp.tile([1, 512], F32, tag="z", name="z_sb")
            nc.scalar.copy(z_sb[:], z_ps[0:1, :])
            rzt_ps = stp.tile([128, 4], F32, tag="st", name="rzt_ps")
            for k in range(4):
                nc.tensor.transpose(
                    rzt_ps[:, k : k + 1],
                    z_sb[0:1, ts(k, 128)],
                    id32_sb[:],
                )
            rzt = rzp.tile([128, 4], F32, tag="rzt", name="rzt")
            nc.vector.reciprocal(rzt[:], rzt_ps[:])
            for k in range(4):
                nc.vector.tensor_scalar_mul(
                    o_big[:, ts(k, 512)], o_big[:, ts(k, 512)], rzt[:, k : k + 1]
                )
                r0 = 512 * s + 128 * k
                nc.sync.dma_start(out[r0 : r0 + 128, :], o_big[:, ts(k, 512)])


def build_nc():
    nc = bacc.Bacc(
        "TRN2",
        target_bir_lowering=False,
        debug=False,
        enable_asserts=False,
        num_devices=B,
    )
    with tile.TileContext(nc) as tc:
        _emit(tc)
    nc.compile()
    return nc


def host_constants():
    """consts blob: [:, 0:128] causal triangle, [:, 128:260] ones,
    [0, 260] the 1x1 identity used by the tiny Z transposes."""
    p = np.arange(128)[:, None]
    f = np.arange(128)[None, :]
    consts = np.zeros((128, 261), dtype=np.float32)
    consts[:, 0:128] = (p <= f).astype(np.float32)
    consts[:, 128:260] = 1.0
    consts[0, 260] = 1.0
    return consts


_nc_cache = {}


def _install_ntff_hook():
    """Provide antenv.axon_hooks (absent in this image) so that
    run_bass_kernel_spmd(trace=True) can capture NTFF profiles via the
    axon ctypes hook from trn_agent_boot."""
    import types

    if "antenv.axon_hooks" in sys.modules:
        return
    mod = types.ModuleType("antenv.axon_hooks")
    holder = [None]
    mod.set_axon_ntff_profile_hook = lambda h: holder.__setitem__(0, h)
    mod.get_axon_ntff_profile_hook = lambda: holder[0]
    sys.modules["antenv.axon_hooks"] = mod
    try:
        from trn_agent_boot.trn_boot import _ntff_profile_via_ctypes

        holder[0] = _ntff_profile_via_ctypes("/opt/axon/libaxon_pjrt.so")
    except Exception as e:  # degrade to no tracing
        print(f"ntff hook install failed: {e}", file=sys.stderr)


def kernel(x, Wq, Wk, Wv):
    from concourse import bass_utils

    x = np.ascontiguousarray(np.asarray(x, dtype=np.float32))
    Wq = np.ascontiguousarray(np.asarray(Wq, dtype=np.float32))
    Wk = np.ascontiguousarray(np.asarray(Wk, dtype=np.float32))
    Wv = np.ascontiguousarray(np.asarray(Wv, dtype=np.float32))
    assert x.shape == (B, T, C_IN), x.shape

    if "nc" not in _nc_cache:
        _nc_cache["nc"] = build_nc()
    nc = _nc_cache["nc"]

    consts = host_constants()
    z = np.zeros_like(Wq)
    wqk = np.ascontiguousarray(np.concatenate([Wq, z, Wk, z], axis=1))
    in_maps = [
        {
            "xt": np.ascontiguousarray(x[b].T),
            "wqk": wqk,
            "wv": Wv,
            "consts": consts,
        }
        for b in range(B)
    ]
    trace = os.environ.get("KERNEL_TRACE", "0") == "1"
    if trace:
        _install_ntff_hook()
    res = bass_utils.run_bass_kernel_spmd(
        nc, in_maps, core_ids=list(range(B)), trace=trace
    )
    global last_result
    last_result = res
    return np.stack([r["out"] for r in res.results], axis=0)


# revision 29
# speedup vs baseline: 1.7577x; 1.1227x over previous
"""Causal single-head attention (B=8, T=2048, C=512, D=64) on 8 trn2 NeuronCores.

Sharding: data-parallel over batch — core b computes the full causal attention
for x[b]; the small projection weights are replicated to every core. No
collectives are needed, and the final output is gathered on the host by
stacking the 8 per-core results.

Host-side prep (layout only, no FLOPs): x[b] is passed pre-transposed as
xT [C, T] (saves 64 PE transposes per core), Wq/Wk are zero-padded to
[C, 128] and fused into one wqk tensor (K=64 matmuls stream the moving
operand at half rate on trn2 — padding the contraction to 128 with zeros
doubles ST matmul throughput), and the small constants (causal triangle,
ones, identity scalar) ride in one blob to minimize DMA issue ops.

Per-core dataflow (fp32 storage, float32r matmuls, all resident in SBUF):
  1. QT/KT [128, T] = wqk_chunk.T @ xT_chunk   (rows 64:128 are zeros)
     V [t, c] tiles  = xT_chunk.T @ Wv_chunk   (natural layout for step 3)
  2. per query slice s (512 wide), per key chunk j (128):
       ST [tk=128, tq=512] = matmul(lhsT=KT[:, j], rhs=QT[:, s])  (PSUM)
       E  = exp(0.125 * ST)    ACT, PSUM->SBUF, restricted to the causally
                               valid columns; band chunks multiply the
                               [128,128] diagonal block by the triangle mask.
                               No max-subtraction: scores ~ N(0,1), exp of a
                               few units cannot overflow fp32.
       Z[tq] += ones.T @ E     row sums on the PE (1 matmul per chunk)
       out[tq=128, c=512] += matmul(lhsT=E[:, k*128:...], rhs=V_j)
     The second matmul lands the output in natural [t, c] layout directly,
     so no output transposes are needed. Outputs are copied out of PSUM
     unnormalized as soon as each 128-row block's accumulation finishes
     (frees the PSUM bank early), then scaled by 1/Z (transposed to a
     per-partition column via a tiny PE transpose, reciprocal on DVE) and
     DMA'd per 128-row block.

Performance notes (measured on trn2 via NTFF profiles):
  - float32r runs 1 cycle/column only when the operand APs are strided
    (a fully contiguous AP or a K=64 operand drops to 2 cycles/column) —
    hence the zero-padded K and the deliberate 516/132-wide tile padding.
  - ST matmuls are software-pipelined DEPTH=3 ahead so the PE never waits
    on the ACT exp; input DMAs are batched and ordered by first use across
    both HWDGE queues.
"""

import os
import sys

if "/opt/trn_rl_repo" not in sys.path:
    sys.path.insert(0, "/opt/trn_rl_repo")

import numpy as np

import concourse.tile as tile
from concourse import bacc, mybir
from concourse.bass import ts

B, T, C_IN, C_OUT, D = 8, 2048, 512, 512, 64
NT = T // 128  # 16 key chunks / t tiles
NC = C_IN // 128  # 4 c_in chunks
NS = T // 512  # 4 query slices
F32 = mybir.dt.float32
F32R = mybir.dt.float32r

USE_F32R = os.environ.get("KERNEL_F32R", "1") == "1"
MMDT = F32R if USE_F32R else F32

last_result = None  # BassKernelResults of the most recent run (for test harness)


def _emit(tc):
    nc = tc.nc

    xt = nc.dram_tensor("xt", [C_IN, T], MMDT, kind="ExternalInput").ap()
    wqk = nc.dram_tensor("wqk", [C_IN, 256], MMDT, kind="ExternalInput").ap()
    wv = nc.dram_tensor("wv", [C_IN, C_OUT], MMDT, kind="ExternalInput").ap()
    # consts blob: [:, 0:128] tri, [:, 128:260] ones, [0, 260] identity scalar
    consts = nc.dram_tensor("consts", [128, 261], MMDT, kind="ExternalInput").ap()
    out = nc.dram_tensor("out", [T, C_OUT], F32, kind="ExternalOutput").ap()

    with (
        tc.tile_pool(name="persist", bufs=1) as pp,
        tc.tile_pool(name="epool", bufs=4) as ep,
        tc.tile_pool(name="opool", bufs=2) as outp,
        tc.tile_pool(name="rzp", bufs=2) as rzp,
        tc.tile_pool(name="stp", bufs=3, space="PSUM") as stp,
        tc.tile_pool(name="op", bufs=4, space="PSUM") as op,
        tc.tile_pool(name="zp", bufs=1, space="PSUM") as zp,
    ):
        # ---- persistent SBUF tensors ----
        xt_sb = pp.tile([128, NC * T], MMDT, tag="xt")  # c-chunk j at ts(j, 2048)
        qt_sb = pp.tile([128, T], MMDT, tag="qt")
        kt_sb = pp.tile([128, T], MMDT, tag="kt")
        v_sb = pp.tile([128, NT * C_OUT], MMDT, tag="v")  # tk-tile j at ts(j, 512)
        wqk_sb = pp.tile([128, NC * 256], MMDT, tag="wqk")
        wv_sb = pp.tile([128, NC * C_OUT], MMDT, tag="wv")
        consts_sb = pp.tile([128, 261], MMDT, tag="consts")
        tri_sb = consts_sb[:, 0:128]
        ones_sb = consts_sb[:, 128:260]
        id32_sb = consts_sb[0:1, 260:261].bitcast(F32)

        # ---- input DMAs: batched, ordered by first-use time ----
        nc.sync.dma_start(xt_sb[:, 0:512], xt[0:128, 0:512])
        nc.scalar.dma_start(
            wqk_sb.rearrange("p (j d) -> p j d", d=256),
            wqk.rearrange("(j p) d -> p j d", p=128),
        )
        nc.sync.dma_start(xt_sb[:, 4096:4608], xt[256:384, 0:512])
        nc.scalar.dma_start(xt_sb[:, 2048:2560], xt[128:256, 0:512])
        nc.sync.dma_start(wv_sb[:, 0:512], wv[0:128, :])
        nc.scalar.dma_start(xt_sb[:, 6144:6656], xt[384:512, 0:512])
        nc.sync.dma_start(wv_sb[:, 512:1024], wv[128:256, :])
        nc.sync.dma_start(wv_sb[:, 1024:1536], wv[256:384, :])
        nc.sync.dma_start(wv_sb[:, 1536:2048], wv[384:512, :])
        nc.scalar.dma_start(consts_sb[:], consts)
        for j in range(NC):
            eng = nc.sync if j % 2 == 0 else nc.scalar
            eng.dma_start(
                xt_sb[:, 2048 * j + 512 : 2048 * (j + 1)],
                xt[128 * j : 128 * (j + 1), 512:2048],
            )

        # ---- projections, per t-group g ----
        for g in range(4):
            s = g
            q_ps = stp.tile([128, 512], F32, tag="st", name="q_ps")
            for j in range(NC):
                nc.tensor.matmul(
                    q_ps[:],
                    wqk_sb[:, 256 * j : 256 * j + 128],
                    xt_sb[:, 2048 * j + 512 * s : 2048 * j + 512 * (s + 1)],
                    start=(j == 0),
                    stop=(j == NC - 1),
                )
            nc.vector.tensor_copy(qt_sb[:, ts(s, 512)], q_ps[:])
            k_ps = stp.tile([128, 512], F32, tag="st", name="k_ps")
            for j in range(NC):
                nc.tensor.matmul(
                    k_ps[:],
                    wqk_sb[:, 256 * j + 128 : 256 * (j + 1)],
                    xt_sb[:, 2048 * j + 512 * s : 2048 * j + 512 * (s + 1)],
                    start=(j == 0),
                    stop=(j == NC - 1),
                )
            nc.vector.tensor_copy(kt_sb[:, ts(s, 512)], k_ps[:])
            for i in range(4 * g, 4 * g + 4):
                v_ps = stp.tile([128, 512], F32, tag="st", name="v_ps")
                for j in range(NC):
                    nc.tensor.matmul(
                        v_ps[:],
                        xt_sb[:, 2048 * j + 128 * i : 2048 * j + 128 * (i + 1)],
                        wv_sb[:, ts(j, 512)],
                        start=(j == 0),
                        stop=(j == NC - 1),
                    )
                nc.vector.tensor_copy(v_sb[:, ts(i, 512)], v_ps[:])

        # ---- attention ----
        def emit_st(s, j):
            st_ps = stp.tile([128, 512], F32, tag="st", name="st_ps")
            nc.tensor.matmul(
                st_ps[:],
                kt_sb[:, ts(j, 128)],
                qt_sb[:, ts(s, 512)],
                start=True,
                stop=True,
            )
            return st_ps

        DEPTH = 3
        pend = {0: {j: emit_st(0, j) for j in range(DEPTH)}}
        o_bigs = {}
        for s in range(NS):
            nj = 4 * s + 4
            st_tiles = pend.pop(s)
            o_ps = [
                op.tile([128, 512], F32, tag="o", name=f"o_ps{k}") for k in range(4)
            ]
            z_ps = zp.tile([1, 512], F32, name="z_ps")
            o_bigs[s] = outp.tile([128, 2048], F32, name="o_big")
            for j in range(nj):
                jn = j + DEPTH
                if jn < nj:
                    st_tiles[jn] = emit_st(s, jn)
                elif s + 1 < NS and jn - nj < DEPTH:
                    pend.setdefault(s + 1, {})[jn - nj] = emit_st(s + 1, jn - nj)
                st_ps = st_tiles.pop(j)
                e = ep.tile([128, 516], MMDT, name="e")
                r = j - 4 * s  # band index; valid key rows start at column 128*r
                lo = 128 * r if r >= 0 else 0
                nc.scalar.activation(
                    e[:, lo:512],
                    st_ps[:, lo:512],
                    mybir.ActivationFunctionType.Exp,
                    scale=0.125,
                )
                if r >= 0:
                    nc.vector.tensor_mul(e[:, ts(r, 128)], e[:, ts(r, 128)], tri_sb[:])
                nc.tensor.matmul(
                    z_ps[0:1, lo:512],
                    ones_sb[:, 0:1],
                    e[:, lo:512],
                    start=(j == 0),
                    stop=(j == nj - 1),
                )
                o_big = o_bigs[s]
                for k in range(4):
                    m = 4 * s + k
                    if j <= m:
                        nc.tensor.matmul(
                            o_ps[k][:],
                            e[:, ts(k, 128)],
                            v_sb[:, ts(j, 512)],
                            start=(j == 0),
                            stop=(j == m),
                        )
                        if j == m:
                            # accumulation done: copy out unnormalized now so
                            # the PSUM bank frees before the 1/Z chain finishes
                            nc.scalar.copy(o_big[:, ts(k, 512)], o_ps[k][:])
            # 1/Z: copy row to SBUF (ACT), transpose to columns (PE), recip (DVE)
            z_sb = rzp.tile([1, 512], F32, tag="z", name="z_sb")
            nc.scalar.copy(z_sb[:], z_ps[0:1, :])
            rzt_ps = stp.tile([128, 4], F32, tag="st", name="rzt_ps")
            for k in range(4):
                nc.tensor.transpose(
                    rzt_ps[:, k : k + 1],
                    z_sb[0:1, ts(k, 128)],
                    id32_sb[:],
                )
            rzt = rzp.tile([128, 4], F32, tag="rzt", name="rzt")
            nc.vector.reciprocal(rzt[:], rzt_ps[:])
            for k in range(4):
                # split the normalize muls across DVE and ACT so the epilogue
                # chain (and the kernel tail) runs them two at a time
                if k % 2 == 0:
                    nc.vector.tensor_scalar_mul(
                        o_big[:, ts(k, 512)], o_big[:, ts(k, 512)], rzt[:, k : k + 1]
                    )
                else:
                    nc.scalar.activation(
                        o_big[:, ts(k, 512)],
                        o_big[:, ts(k, 512)],
                        mybir.ActivationFunctionType.Copy,
                        scale=rzt[:, k : k + 1],
                    )
                r0 = 512 * s + 128 * k
                nc.sync.dma_start(out[r0 : r0 + 128, :], o_big[:, ts(k, 512)])


def build_nc():
    nc = bacc.Bacc(
        "TRN2",
        target_bir_lowering=False,
        debug=False,
        enable_asserts=False,
        num_devices=B,
    )
    with tile.TileContext(nc) as tc:
        _emit(tc)
    nc.compile()
    return nc


def host_constants():
    """consts blob: [:, 0:128] causal triangle, [:, 128:260] ones,
    [0, 260] the 1x1 identity used by the tiny Z transposes."""
    p = np.arange(128)[:, None]
    f = np.arange(128)[None, :]
    consts = np.zeros((128, 261), dtype=np.float32)
    consts[:, 0:128] = (p <= f).astype(np.float32)
    consts[:, 128:260] = 1.0
    consts[0, 260] = 1.0
    return consts


_nc_cache = {}


def _install_ntff_hook():
    """Provide antenv.axon_hooks (absent in this image) so that
    run_bass_kernel_spmd(trace=True) can capture NTFF profiles via the
    axon ctypes hook from trn_agent_boot."""
    import types

    if "antenv.axon_hooks" in sys.modules:
        return
    mod = types.ModuleType("antenv.axon_hooks")
    holder = [None]
    mod.set_axon_ntff_profile_hook = lambda h: holder.__setitem__(0, h)
    mod.get_axon_ntff_profile_hook = lambda: holder[0]
    sys.modules["antenv.axon_hooks"] = mod
    try:
        from trn_agent_boot.trn_boot import _ntff_profile_via_ctypes

        holder[0] = _ntff_profile_via_ctypes("/opt/axon/libaxon_pjrt.so")
    except Exception as e:  # degrade to no tracing
        print(f"ntff hook install failed: {e}", file=sys.stderr)


def kernel(x, Wq, Wk, Wv):
    from concourse import bass_utils

    x = np.ascontiguousarray(np.asarray(x, dtype=np.float32))
    Wq = np.ascontiguousarray(np.asarray(Wq, dtype=np.float32))
    Wk = np.ascontiguousarray(np.asarray(Wk, dtype=np.float32))
    Wv = np.ascontiguousarray(np.asarray(Wv, dtype=np.float32))
    assert x.shape == (B, T, C_IN), x.shape

    if "nc" not in _nc_cache:
        _nc_cache["nc"] = build_nc()
    nc = _nc_cache["nc"]

    consts = host_constants()
    z = np.zeros_like(Wq)
    wqk = np.ascontiguousarray(np.concatenate([Wq, z, Wk, z], axis=1))
    in_maps = [
        {
            "xt": np.ascontiguousarray(x[b].T),
            "wqk": wqk,
            "wv": Wv,
            "consts": consts,
        }
        for b in range(B)
    ]
    trace = os.environ.get("KERNEL_TRACE", "0") == "1"
    if trace:
        _install_ntff_hook()
    res = bass_utils.run_bass_kernel_spmd(
        nc, in_maps, core_ids=list(range(B)), trace=trace
    )
    global last_result
    last_result = res
    return np.stack([r["out"] for r in res.results], axis=0)
